# revision 8
# baseline (speedup 1.0000x reference)
"""ARAP gradient kernel for 8 TRN2 NeuronCores.

Vertex-sharded: core r owns vertices [12500r, 12500(r+1)) for ALL 8 batches.
Each core builds a bf16 feature table slice [p1(3), t(3), R(9), 1] x 8 batches
(256B per vertex row), AllGathers the full table, then gathers its 200K edges'
neighbor rows with dma_gather (256B descriptors), weights them, and reduces
per-vertex with PE block-diagonal matmuls accumulating in PSUM across the four
int16-index chunk passes. A final per-vertex pass combines the reduced sums:

  g_i = aw * (2*W*p2_i - R_i(W*p1_i - S1_i) - SR_i*p1_i + St_i)

where S1 = sum w*p1_j, St = sum w*(R_j p1_j - 2 p2_j), SR = sum w*R_j,
W = sum w (from the constant-1 feature slot).
"""

import numpy as np

B = 8
N = 100000
K = 16
NCORES = 8
VREAL = N // NCORES          # 12500 real vertices per core
VPC = 12800                  # padded vertices per core (= 128*100)
QCOL = 100                   # columns per partition in vertex-major layouts
TROWS = NCORES * VPC         # 102400 global table rows
F = 128                      # 8 batches * 16 bf16 feature slots
CHUNK = 32768
NCH = 4
MQ = 32                      # gather columns per tile
NT = (VPC * K // 128) // MQ  # 50 tiles (1600 columns total)
NIDX_T = MQ * 128            # 4096 indices per tile-pass
NSPLIT = 4                   # dma_gathers per tile-pass (128-desc ring cap)
MQS = MQ // NSPLIT
NIDX_S = NIDX_T // NSPLIT

_cache = {}


def _build():
    from concourse import bass, bacc, mybir
    from concourse.tile import TileContext

    nc = bacc.Bacc(None)
    dt = mybir.dt

    xyz1_p = nc.declare_dram_parameter("xyz1s", [B, VPC, 3], dt.float32, isOutput=False)
    xyz2_p = nc.declare_dram_parameter("xyz2s", [B, VPC, 3], dt.float32, isOutput=False)
    rot_p = nc.declare_dram_parameter("rots", [B, VPC, 9], dt.float32, isOutput=False)
    idx_p = nc.declare_dram_parameter("idxw", [NCH, NT, NSPLIT, 128, NIDX_S // 16], dt.int16, isOutput=False)
    w_p = nc.declare_dram_parameter("wts", [NCH, NT, 128, MQ], dt.float32, isOutput=False)
    ones_p = nc.declare_dram_parameter("ones16", [128, 8], dt.bfloat16, isOutput=False)
    aw_p = nc.declare_dram_parameter("aw", [128, 1], dt.float32, isOutput=False)
    g_p = nc.declare_dram_parameter("g", [B, VPC, 3], dt.float32, isOutput=True)

    myT = nc.dram_tensor("myT", [VPC, F], dt.bfloat16)
    T_all = nc.dram_tensor("T_all", [TROWS, F], dt.bfloat16, addr_space="Shared")
    S_dram = nc.dram_tensor("S_dram", [VPC, F], dt.float32)

    with TileContext(nc) as tc:
        with (
            tc.tile_pool(name="sbuf", bufs=2) as pool,
            tc.tile_pool(name="feat", bufs=1) as fpool,
            tc.tile_pool(name="gat", bufs=3) as gpool,
            tc.tile_pool(name="psum", bufs=1, space="PSUM") as ppool,
        ):
            ones_t = fpool.tile([128, 8], dt.bfloat16, tag="ones")
            nc.sync.dma_start(out=ones_t[:], in_=ones_p[:, :])
            aw_t = fpool.tile([128, 1], dt.float32, tag="aw")
            nc.sync.dma_start(out=aw_t[:], in_=aw_p[:, :])

            # ---- Phase A: feature table slice -------------------------------
            FS = fpool.tile([128, QCOL, F], dt.bfloat16, tag="FS")
            nc.vector.memset(FS[:, :, :], 1.0)
            for b in range(B):
                p1 = pool.tile([128, QCOL, 3], dt.float32, tag="p1")
                p2 = pool.tile([128, QCOL, 3], dt.float32, tag="p2")
                R = pool.tile([128, QCOL, 9], dt.float32, tag="R")
                nc.sync.dma_start(out=p1[:], in_=xyz1_p[b].rearrange("(p q) c -> p q c", p=128))
                nc.sync.dma_start(out=p2[:], in_=xyz2_p[b].rearrange("(p q) c -> p q c", p=128))
                nc.sync.dma_start(out=R[:], in_=rot_p[b].rearrange("(p q) c -> p q c", p=128))
                fo = b * 16
                # p1 -> slots 0:3
                nc.vector.tensor_copy(out=FS[:, :, fo + 0 : fo + 3], in_=p1[:, :, :])
                # t = R @ p1 - 2*p2 -> slots 3:6
                for a in range(3):
                    acc = pool.tile([128, QCOL], dt.float32, tag="acc")
                    tmp = pool.tile([128, QCOL], dt.float32, tag="tmp")
                    nc.vector.tensor_tensor(out=acc[:], in0=R[:, :, 3 * a], in1=p1[:, :, 0], op=mybir.AluOpType.mult)
                    nc.vector.tensor_tensor(out=tmp[:], in0=R[:, :, 3 * a + 1], in1=p1[:, :, 1], op=mybir.AluOpType.mult)
                    nc.vector.tensor_tensor(out=acc[:], in0=acc[:], in1=tmp[:], op=mybir.AluOpType.add)
                    nc.vector.tensor_tensor(out=tmp[:], in0=R[:, :, 3 * a + 2], in1=p1[:, :, 2], op=mybir.AluOpType.mult)
                    nc.vector.tensor_tensor(out=acc[:], in0=acc[:], in1=tmp[:], op=mybir.AluOpType.add)
                    nc.vector.tensor_scalar_mul(out=tmp[:], in0=p2[:, :, a], scalar1=-2.0)
                    nc.vector.tensor_tensor(out=FS[:, :, fo + 3 + a], in0=acc[:], in1=tmp[:], op=mybir.AluOpType.add)
                # R -> slots 6:15  (slot 15 stays 1.0 from memset)
                nc.vector.tensor_copy(out=FS[:, :, fo + 6 : fo + 15], in_=R[:, :, :])
            nc.sync.dma_start(out=myT.rearrange("(p q) f -> p q f", p=128), in_=FS[:, :, :])

            # ---- Phase B: AllGather -----------------------------------------
            nc.gpsimd.collective_compute(
                "AllGather",
                mybir.AluOpType.bypass,
                replica_groups=[list(range(NCORES))],
                ins=[myT[:]],
                outs=[T_all[:]],
            )

            # ---- Phase C: gather + weighted segment reduce ------------------
            for t in range(NT):
                pss = [ppool.tile([8, 512], dt.float32, name=f"ps{qq}_{t}", tag=f"ps{qq}") for qq in range(8)]
                for c in range(NCH):
                    idx_t = gpool.tile([128, NSPLIT, NIDX_S // 16], dt.int16, tag="idx")
                    nc.sync.dma_start(out=idx_t[:], in_=idx_p[c, t].rearrange("sp p q -> p sp q"))
                    g_t = gpool.tile([128, MQ, F], dt.bfloat16, tag="g")
                    base = CHUNK * c
                    hi = min(CHUNK, TROWS - base)
                    for sp in range(NSPLIT):
                        nc.gpsimd.dma_gather(
                            out_ap=g_t[:, sp * MQS : (sp + 1) * MQS, :],
                            in_ap=T_all[base : base + hi, :],
                            idxs_ap=idx_t[:, sp, :],
                            num_idxs=NIDX_S,
                            num_idxs_reg=NIDX_S,
                            elem_size=F,
                        )
                    w_t = gpool.tile([128, MQ], dt.float32, tag="w")
                    nc.sync.dma_start(out=w_t[:], in_=w_p[c, t].rearrange("p q -> p q"))
                    wbf = gpool.tile([128, MQ], dt.bfloat16, tag="wbf")
                    nc.vector.tensor_copy(out=wbf[:], in_=w_t[:])
                    wg = gpool.tile([128, MQ, F], dt.bfloat16, tag="wg")
                    nc.vector.tensor_tensor(
                        out=wg[:, :, :],
                        in0=g_t[:, :, :],
                        in1=wbf[:, :, None].to_broadcast([128, MQ, F]),
                        op=mybir.AluOpType.mult,
                    )
                    for qq in range(8):
                        nc.tensor.matmul(
                            out=pss[qq][:, :],
                            lhsT=ones_t[:, :],
                            rhs=wg[:, 4 * qq : 4 * qq + 4, :],
                            start=(c == 0),
                            stop=(c == NCH - 1),
                        )
                St = pool.tile([8, 8, 4, 128], dt.float32, tag="St")
                for qq in range(8):
                    nc.scalar.copy(out=St[:, qq, :, :], in_=pss[qq][:, :])
                # S row = 256*t + 32*qq + 8*q2 + s
                dst = S_dram[256 * t : 256 * (t + 1)].rearrange("(qq q2 s) f -> s qq q2 f", qq=8, q2=4)
                nc.sync.dma_start(out=dst, in_=St[:, :, :, :])

            # ---- Phase D: per-vertex combine --------------------------------
            S = fpool.tile([128, QCOL, F], dt.float32, tag="S")
            nc.sync.dma_start(out=S[:], in_=S_dram.rearrange("(p q) f -> p q f", p=128))
            awb = aw_t[:, :].to_broadcast([128, QCOL])
            for b in range(B):
                p1 = pool.tile([128, QCOL, 3], dt.float32, tag="p1")
                p2 = pool.tile([128, QCOL, 3], dt.float32, tag="p2")
                R = pool.tile([128, QCOL, 9], dt.float32, tag="R")
                nc.sync.dma_start(out=p1[:], in_=xyz1_p[b].rearrange("(p q) c -> p q c", p=128))
                nc.sync.dma_start(out=p2[:], in_=xyz2_p[b].rearrange("(p q) c -> p q c", p=128))
                nc.sync.dma_start(out=R[:], in_=rot_p[b].rearrange("(p q) c -> p q c", p=128))
                fo = b * 16
                W = S[:, :, fo + 15]
                gout = pool.tile([128, QCOL, 3], dt.float32, tag="gout")
                u = pool.tile([128, QCOL, 3], dt.float32, tag="u")
                # u = W*p1 - S1
                for a in range(3):
                    tmp = pool.tile([128, QCOL], dt.float32, tag="tmp")
                    nc.vector.tensor_tensor(out=tmp[:], in0=W, in1=p1[:, :, a], op=mybir.AluOpType.mult)
                    nc.vector.tensor_tensor(out=u[:, :, a], in0=tmp[:], in1=S[:, :, fo + a], op=mybir.AluOpType.subtract)
                for a in range(3):
                    acc = pool.tile([128, QCOL], dt.float32, tag="acc")
                    tmp = pool.tile([128, QCOL], dt.float32, tag="tmp")
                    # acc = R_i @ u   (row a)
                    nc.vector.tensor_tensor(out=acc[:], in0=R[:, :, 3 * a], in1=u[:, :, 0], op=mybir.AluOpType.mult)
                    nc.vector.tensor_tensor(out=tmp[:], in0=R[:, :, 3 * a + 1], in1=u[:, :, 1], op=mybir.AluOpType.mult)
                    nc.vector.tensor_tensor(out=acc[:], in0=acc[:], in1=tmp[:], op=mybir.AluOpType.add)
                    nc.vector.tensor_tensor(out=tmp[:], in0=R[:, :, 3 * a + 2], in1=u[:, :, 2], op=mybir.AluOpType.mult)
                    nc.vector.tensor_tensor(out=acc[:], in0=acc[:], in1=tmp[:], op=mybir.AluOpType.add)
                    # acc += SR @ p1  (row a)
                    for j in range(3):
                        nc.gpsimd.tensor_tensor(out=tmp[:], in0=S[:, :, fo + 6 + 3 * a + j], in1=p1[:, :, j], op=mybir.AluOpType.mult)
                        nc.vector.tensor_tensor(out=acc[:], in0=acc[:], in1=tmp[:], op=mybir.AluOpType.add)
                    # gout_a = 2*W*p2_a - acc + St_a
                    nc.gpsimd.tensor_tensor(out=tmp[:], in0=W, in1=p2[:, :, a], op=mybir.AluOpType.mult)
                    nc.vector.tensor_scalar_mul(out=tmp[:], in0=tmp[:], scalar1=2.0)
                    nc.vector.tensor_tensor(out=tmp[:], in0=tmp[:], in1=acc[:], op=mybir.AluOpType.subtract)
                    nc.vector.tensor_tensor(out=tmp[:], in0=tmp[:], in1=S[:, :, fo + 3 + a], op=mybir.AluOpType.add)
                    nc.vector.tensor_tensor(out=gout[:, :, a], in0=tmp[:], in1=awb, op=mybir.AluOpType.mult)
                nc.sync.dma_start(out=g_p[b].rearrange("(p q) c -> p q c", p=128), in_=gout[:])

    nc.compile()
    return nc


def _host_prep(xyz1, xyz2, neighborList, numNeighbors, accnumNeighbors, weightMatrix, rotations, arapWeight):
    """Index/layout-only preprocessing. Returns per-core input maps."""
    nbr = np.asarray(neighborList).astype(np.int64)
    wm = np.asarray(weightMatrix).astype(np.float32)
    # global table row of vertex j
    rows = VPC * (nbr // VREAL) + (nbr % VREAL)

    ones16 = np.zeros((128, 8), np.float32)
    for p in range(128):
        ones16[p, p // 16] = 1.0
    import jax.numpy as jnp
    ones16 = np.asarray(jnp.asarray(ones16, jnp.bfloat16))

    in_maps = []
    for r in range(NCORES):
        v0 = r * VREAL
        xyz1s = np.zeros((B, VPC, 3), np.float32)
        xyz2s = np.zeros((B, VPC, 3), np.float32)
        rots = np.zeros((B, VPC, 9), np.float32)
        xyz1s[:, :VREAL] = xyz1[:, v0 : v0 + VREAL]
        xyz2s[:, :VREAL] = xyz2[:, v0 : v0 + VREAL]
        rots[:, :VREAL] = np.asarray(rotations[:, v0 : v0 + VREAL]).reshape(B, VREAL, 9)

        # slot (p, q) within tile t: vertex v = 512*t + 32*qq + 4*s + q2,
        # k = p % 16, with q = 64*t? (local col j = 4*qq + q2), s = p//16.
        # Build flat per-(tile) edge index array in position order pos = j*128 + p.
        rows_loc = np.zeros((NT, MQ * 128), np.int64)
        w_loc = np.zeros((NT, 128, MQ), np.float32)
        t_i = np.arange(NT)[:, None, None]
        j_i = np.arange(MQ)[None, :, None]
        p_i = np.arange(128)[None, None, :]
        qq = j_i // 4
        q2 = j_i % 4
        s = p_i // 16
        kk = p_i % 16
        v_loc = 256 * t_i + 32 * qq + 8 * q2 + s          # [NT, MQ, 128]
        valid = v_loc < VREAL
        vg = np.where(valid, v_loc + v0, 0)
        e_ids = vg * K + kk
        rws = np.where(valid, rows[e_ids], 0)
        wvals = np.where(valid, wm[e_ids], 0.0)
        rows_loc = rws.reshape(NT, MQ * 128)
        # weights laid out [128, MQ]
        w_all = np.transpose(wvals, (0, 2, 1))            # [NT, 128, MQ]

        idxw = np.zeros((NCH, NT, NSPLIT, 128, NIDX_S // 16), np.int16)
        wts = np.zeros((NCH, NT, 128, MQ), np.float32)
        for c in range(NCH):
            base = CHUNK * c
            hi = min(CHUNK, TROWS - base)
            rel = np.clip(rows_loc - base, 0, hi - 1).astype(np.int16)   # [NT, 8192]
            inch = (rows_loc >= base) & (rows_loc < base + hi)
            # wrapped-16, replicated to 128 partitions: pos = f*16 + l
            relw = rel.reshape(NT, NSPLIT, NIDX_S // 16, 16)              # [NT, sp, f, l]
            idxw[c] = np.tile(np.transpose(relw, (0, 1, 3, 2)), (1, 1, 8, 1))
            msk = np.transpose(inch.reshape(NT, MQ, 128), (0, 2, 1))
            wts[c] = w_all * msk

        in_maps.append({
            "xyz1s": xyz1s, "xyz2s": xyz2s, "rots": rots,
            "idxw": idxw, "wts": wts, "ones16": ones16,
            "aw": np.full((128, 1), np.float32(arapWeight)),
        })
    return in_maps


def _execute(in_maps, trace=False, **kw):
    from concourse.bass_utils import run_bass_kernel_spmd
    if "nc" not in _cache:
        _cache["nc"] = _build()
    return run_bass_kernel_spmd(_cache["nc"], in_maps, list(range(NCORES)), trace=trace, **kw)


def kernel(**inputs):
    in_maps = _host_prep(**inputs)
    res = _execute(in_maps)
    out = np.concatenate([res.results[r]["g"][:, :VREAL] for r in range(NCORES)], axis=1)
    return out.astype(np.float32)
